# revision 38
# baseline (speedup 1.0000x reference)
"""Trainium2 Bass kernel for nn_Polynomial_91259465105963 (gnn_message_passing).

8 NeuronCores, to-sharded: core c owns to-nodes J_c=[16c,16c+16). Key
structure exploited:
  * complete graph + one-hot features collapse tp1 into per-to-node
    (50 -> 15) matmuls;
  * the 5 permutations are the first 5 lex perms of range(127): they differ
    ONLY at positions {124,125,126}. So the device computes a single
    perm-INDEPENDENT pass (identity weights); the three perm-varying
    to-nodes are handled exactly on the host in f64 (their emb columns are
    zeroed on device, their Y rows zeroed in the host readout).
  * fp16 single-pass matmuls everywhere (PE runs fp16 at bf16 rate, f32
    accumulation in PSUM); emulated end-to-end rel-err ~5e-4 vs tolerance
    2e-2 -- no hi/lo splits needed.
  * the device stops at scal = h @ W (the per-edge tp1 coefficients): the
    Y-multiply + slot reduction + NormActivation + tp2 readout are only
    ~1.5 MFLOP, done on host in f64. On-device those cost ~2.7us of
    DVE/Pool serial chain; shipping scal (128x256 fp16, 64KB) is faster.

Device pipeline per core (~20 engine instructions, measured ~16.8us where
~13.7us is fixed framework preamble/exit-barriers and DMA latency):
  DMAs: estack halves on two queues (Sync + ACT) so the two A matmuls run
     concurrently; W on GpSimd.
  A: z = w1^T emb as 2 block-diagonal matmuls (M=100: two j-halves stacked
     on output partitions; the second placed in PE row-tiles 64-103 via
     tile_position so it overlaps the first). PSUM z (100, 1024).
  silu: 1 ACT op using the hardware Silu table -> h fp16 (100, 1024).
  B: 8 pair matmuls, lhsT = h (100, 128) slice, rhs = block-diag W
     (100, 32) -> scal PSUM (128, 16 slots x 16ch).
  out: ACT copies scal -> fp16 SBUF, GpSimd DMAs it to HBM.
Host: per-core Y-multiply + slot sum (f64), adds the 3 special to-nodes'
messages per perm, NormActivation + tp2 readout.
"""
import sys
import numpy as np
from itertools import permutations, islice

N = 128
BASIS = 20
MUL = 5
H = 50
D_IN = N + 1
ACT_CONST = 1.6790
C_SMOOTH = 1.14136 * float(np.exp(2.0))
NCORES = 8
JL = N // NCORES              # 16 to-nodes per core
NPAIR = JL // 2               # 8 pair-matmuls
SPECIAL = (124, 125, 126)     # to-nodes whose weights vary across perms

USE_SILU_TABLE = True         # False -> tanh table + DVE stt fallback

_TP2_PATHS = [(0, 0, 2), (2, 1, 1), (2, 1, 3), (3, 2, 0), (3, 2, 2)]
_MOFF = (0, 1, 4, 9)
_MDIM = (1, 3, 5)


def _sh_list(x, y, z):
    s3, s5, s7 = np.sqrt(3.0), np.sqrt(5.0), np.sqrt(7.0)
    s15, s42, s70, s105 = np.sqrt(15.0), np.sqrt(42.0), np.sqrt(70.0), np.sqrt(105.0)
    one = np.ones_like(x)
    y0 = np.stack([one], -1)
    y1 = np.stack([s3 * y, s3 * z, s3 * x], -1)
    y2 = np.stack([s15 * x * y, s15 * y * z, 0.5 * s5 * (3 * z * z - 1.0),
                   s15 * x * z, 0.5 * s15 * (x * x - y * y)], -1)
    y3 = np.stack([0.25 * s70 * y * (3 * x * x - y * y), s105 * x * y * z,
                   0.25 * s42 * y * (5 * z * z - 1.0), 0.5 * s7 * z * (5 * z * z - 3.0),
                   0.25 * s42 * x * (5 * z * z - 1.0), 0.5 * s105 * z * (x * x - y * y),
                   0.25 * s70 * x * (x * x - 3 * y * y)], -1)
    return [y0, y1, y2, y3]


def _gaunt(l1, l2, l3):
    zq, wq = np.polynomial.legendre.leggauss(20)
    nphi = 48
    phi = 2 * np.pi * np.arange(nphi) / nphi
    Z = np.repeat(zq[:, None], nphi, 1)
    P = np.broadcast_to(phi, Z.shape)
    W = np.repeat(wq[:, None], nphi, 1) * (2 * np.pi / nphi)
    st = np.sqrt(np.clip(1.0 - Z * Z, 0.0, None))
    Y = _sh_list(st * np.cos(P), st * np.sin(P), Z)
    G = np.einsum('ab,abi,abj,abk->ijk', W, Y[l1], Y[l2], Y[l3])
    return (G / np.linalg.norm(G)).astype(np.float64)


_CG = [_gaunt(l1, l2, 2) for (_, l1, l2) in _TP2_PATHS]
_PERMS = [list(p) + [N - 1] for p in islice(permutations(range(N - 1)), 5)]

# slot s (0..15) within a core <-> local to-node jl: pair k = s//2,
# half b = s&1 -> jl = k + 8*b. Matches the B-matmul scal column order.
_SLOT_TO_JL = [s // 2 + 8 * (s & 1) for s in range(16)]


# ---------------------------------------------------------------- host prep
def _geom(pos):
    """Per-(i,j) geometry in f64: Ygrid (i,j,16), emb (i,j,20), diag zeroed."""
    f = np.float64
    pos = np.asarray(pos, f)
    dvec = pos[None, :, :] - pos[:, None, :]          # pos[to] - pos[from]
    d2 = np.sum(dvec * dvec, axis=-1)
    np.fill_diagonal(d2, 1.0)
    d = np.sqrt(d2)
    u = dvec / d[..., None]
    Yl = _sh_list(u[..., 0], u[..., 1], u[..., 2])
    Ygrid = np.concatenate(Yl, axis=-1)               # (i, j, 16)
    mask = 1.0 - np.eye(N)
    Ygrid *= mask[:, :, None]
    vals = np.linspace(0.0, 2.0, BASIS + 2)[1:-1]
    step = 2.0 / (BASIS + 1)
    q = (d[..., None] - vals) / step
    g = 1.0 - q * q
    with np.errstate(divide='ignore', over='ignore'):
        emb = np.where(g > 0, np.exp(-2.0 / np.maximum(g, 1e-30)), 0.0) * C_SMOOTH
    emb *= mask[:, :, None]
    return Ygrid, emb


def _wsel_identity(features, fc_w2):
    """Identity-perm weights W[j] (H, 3, 5) incl. tp1 norm + silu consts."""
    f = np.float64
    W2 = np.asarray(fc_w2, f).reshape(H, 3, D_IN, MUL)
    c = (1.0 / np.sqrt(D_IN)) * ACT_CONST / np.sqrt(H)
    if not USE_SILU_TABLE:
        c *= 0.5                      # tanh path: silu = 0.5*z*(1+tanh(z/2))
    A0 = W2[:, :, 0, :]
    feats = np.asarray(features, f)[:, 0]
    Wj = A0[None] * feats[:, None, None, None] + np.moveaxis(W2[:, :, 1:, :], 2, 0)
    return Wj * c                     # (j, H, 3, 5)


def _wsel_perm(features, fc_w2, j, perm_j):
    f = np.float64
    W2 = np.asarray(fc_w2, f).reshape(H, 3, D_IN, MUL)
    c = (1.0 / np.sqrt(D_IN)) * ACT_CONST / np.sqrt(H)
    A0 = W2[:, :, 0, :]
    return (A0 * float(np.asarray(features, f)[j, 0]) + W2[:, :, 1 + perm_j, :]) * c


def _host_prep(pos, features, fc_w1, fc_w2, tp2_w, na_bias):
    f16, f32 = np.float16, np.float32
    Ygrid, emb = _geom(pos)
    w1s = np.asarray(fc_w1, np.float64) / np.sqrt(BASIS)
    Wj = _wsel_identity(features, fc_w2)

    emb_dev = emb.copy()
    emb_dev[:, SPECIAL, :] = 0.0      # specials handled on host

    # w1 block-diagonal (40, 100)
    w1bd = np.zeros((2 * BASIS, 2 * H), np.float64)
    w1bd[0:BASIS, 0:H] = w1s
    w1bd[BASIS:2 * BASIS, H:2 * H] = w1s

    in_maps = []
    for c in range(NCORES):
        jbase = JL * c
        # ---- estack (80, 612): two 40-row blocks (SBUF rows 0-39 / 64-103).
        # Per block: cols 0-99 = w1bd, cols 100-611 = emb columns for pairs
        # 0-3 (block 0) / 4-7 (block 1), col = 100 + 128*(k%4) + i.
        estack = np.zeros((80, 612), f16)
        estack[0:40, 0:100] = w1bd.astype(f16)
        estack[40:80, 0:100] = w1bd.astype(f16)
        for k in range(NPAIR):
            ja = jbase + k
            jb = jbase + k + 8
            rbase = 0 if k < 4 else 40
            cbase = 100 + 128 * (k % 4)
            # emb[i, j, b20] -> rows of estack (basis down rows)
            estack[rbase:rbase + 20, cbase:cbase + 128] = \
                emb_dev[:, ja, :].T.astype(f16)
            estack[rbase + 20:rbase + 40, cbase:cbase + 128] = \
                emb_dev[:, jb, :].T.astype(f16)

        # ---- wstack (100, 256): per pair k cols 32k..32k+32, block-diag
        wstack = np.zeros((100, 256), f16)
        Wflat = Wj.reshape(N, H, 15)
        for k in range(NPAIR):
            ja = jbase + k
            jb = jbase + k + 8
            wstack[0:50, 32 * k:32 * k + 15] = Wflat[ja].astype(f16)
            wstack[50:100, 32 * k + 16:32 * k + 31] = Wflat[jb].astype(f16)

        in_maps.append(dict(
            estack=np.ascontiguousarray(estack),
            wstack=np.ascontiguousarray(wstack),
        ))

    Ydev = Ygrid.copy()
    Ydev[:, SPECIAL, :] = 0.0                       # specials excluded on host
    aux = dict(
        YS=Ygrid.sum(axis=0),                       # (j, 16)
        Ydev=Ydev,                                  # (i, j, 16)
        na_bias=np.asarray(na_bias, np.float64),
        tp2_w=np.asarray(tp2_w, np.float64),
        w1s=w1s,
        features=np.asarray(features, np.float64),
        fc_w2=np.asarray(fc_w2, np.float64),
        emb_special=emb[:, SPECIAL, :],             # (i, 3, 20) exact
        Y_special=Ygrid[:, SPECIAL, :],             # (i, 3, 16)
    )
    return in_maps, aux


# ---------------------------------------------------------------- device emu
def _device_emulate(in_map):
    """Numpy emulation of the device program for one core (fp16 rounding at
    the same places). Returns scal (128, 256) fp16."""
    f16, f32 = np.float16, np.float32
    estack = in_map['estack'].astype(f32)
    wstack = in_map['wstack'].astype(f32)
    # A: two block-diag matmuls (f32 accumulation of fp16 operands)
    z = np.zeros((100, 1024), f32)
    z[:, 0:512] = estack[0:40, 0:100].T @ estack[0:40, 100:612]
    z[:, 512:1024] = estack[40:80, 0:100].T @ estack[40:80, 100:612]
    h = (z / (1.0 + np.exp(-z.astype(np.float64)))).astype(f16).astype(f32)
    # B: 8 pair matmuls -> scal (128, 256)
    scal = np.zeros((N, 256), f32)
    for k in range(NPAIR):
        scal[:, 32 * k:32 * (k + 1)] = h[:, 128 * k:128 * (k + 1)].T @ wstack[:, 32 * k:32 * (k + 1)]
    return scal.astype(f16)


# ---------------------------------------------------------------- bass build
def _build_nc():
    sys.path.insert(0, '/opt/trn_rl_repo')
    import concourse.bass as bass  # noqa: F401
    import concourse.tile as tile
    from concourse import bacc, mybir

    dt = mybir.dt
    f32, f16 = dt.float32, dt.float16
    Alu = mybir.AluOpType
    Act = mybir.ActivationFunctionType

    nc = bacc.Bacc("TRN2", target_bir_lowering=False, debug=False,
                   num_devices=NCORES)
    es_d = nc.dram_tensor("estack", [80, 612], f16, kind="ExternalInput").ap()
    ws_d = nc.dram_tensor("wstack", [100, 256], f16, kind="ExternalInput").ap()
    out_d = nc.dram_tensor("pout", [N, 256], f16, kind="ExternalOutput").ap()

    with tile.TileContext(nc) as tc:
        with tc.tile_pool(name="sb", bufs=1) as sb, \
             tc.tile_pool(name="ps", bufs=1, space="PSUM") as ps:

            # ---- input DMAs. estack halves go to two different queues so
            # both transfers land together and the two A matmuls (distinct
            # PE row-tiles: block 2 lives at SBUF partitions 64-103) run
            # concurrently.
            es = sb.tile([104, 612], f16)
            nc.sync.dma_start(es[0:40, :], es_d[0:40, :])
            nc.scalar.dma_start(es[64:104, :], es_d[40:80, :])
            ws = sb.tile([100, 256], f16)
            nc.gpsimd.dma_start(ws[:], ws_d)

            # ---- A: z = w1bd^T @ emb, two matmuls in distinct PE row-tiles
            zps = ps.tile([100, 1024], f32, tag="zmm")
            nc.tensor.matmul(zps[:, 0:512], es[0:40, 0:100],
                             es[0:40, 100:612], start=True, stop=True)
            nc.tensor.matmul(zps[:, 512:1024], es[64:104, 0:100],
                             es[64:104, 100:612], start=True, stop=True,
                             tile_position=(64, 0))

            # ---- silu -> h fp16 (100, 1024), two ACT ops: the first half
            # starts right after A-mm1 (mm2 lands ~0.35us later), and B's
            # first pair group runs during the second half
            h = sb.tile([100, 1024], f16)
            if USE_SILU_TABLE:
                nc.scalar.activation(h[:, 0:512], zps[:, 0:512], Act.Silu)
                nc.scalar.activation(h[:, 512:1024], zps[:, 512:1024], Act.Silu)
            else:
                t1 = sb.tile([100, 1024], f32, tag="t1")
                nc.scalar.activation(t1[:], zps[:], Act.Tanh, scale=0.5)
                nc.vector.scalar_tensor_tensor(h[:], t1[:], 1.0,
                                               zps[:], Alu.add, Alu.mult)

            # ---- B: 8 pair matmuls -> scal PSUM (128, 16 slots x 16 ch)
            sps = ps.tile([N, 256], f32, tag="smm")
            for k in range(NPAIR):
                nc.tensor.matmul(sps[:, 32 * k:32 * (k + 1)],
                                 h[:, 128 * k:128 * (k + 1)],
                                 ws[:, 32 * k:32 * (k + 1)],
                                 start=True, stop=True)

            # ---- ship scal (fp16) straight to the host: the Y-multiply +
            # slot reduction is only ~1.5 MFLOP there, while on-device it
            # costs ~2.7us of DVE/Pool serial chain. PSUM can't be DMA'd,
            # so copy to fp16 SBUF -- split across ACT and the otherwise
            # idle DVE so the halves run in parallel.
            # Two copy halves: the first (pairs 0-3) overlaps B's second
            # pair group; same-engine (ACT) DMA issue right after -- no
            # cross-engine semaphore hop anywhere on the exit path.
            sc = sb.tile([N, 256], f16)
            nc.scalar.copy(sc[:, 0:128], sps[:, 0:128])
            nc.scalar.copy(sc[:, 128:256], sps[:, 128:256])
            nc.scalar.dma_start(out_d, sc[:])
    nc.compile()
    return nc


# ---------------------------------------------------------------- host finish
def _msg45(scal_lw, Y16):
    """scal_lw (..., 3, 5), Y16 (..., 16) -> (..., 45) [l0 5][l1 15][l2 25]."""
    b0 = scal_lw[..., 0, :] * Y16[..., 0:1]
    b1 = (Y16[..., 1:4, None] * scal_lw[..., None, 1, :]).reshape(*scal_lw.shape[:-2], 15)
    b2 = (Y16[..., 4:9, None] * scal_lw[..., None, 2, :]).reshape(*scal_lw.shape[:-2], 25)
    return np.concatenate([b0, b1, b2], axis=-1)


def _common_from_scals(scals, aux):
    """scals: list of 8 (128, 256) arrays (fp16 from device). Returns the
    perm-independent node features common45 (128, 45) f64."""
    f = np.float64
    Ydev = aux['Ydev']
    common45 = np.zeros((N, 45), f)
    for c in range(NCORES):
        scal_lw = np.asarray(scals[c], f).reshape(N, 16, 16)[:, :, 0:15] \
            .reshape(N, 16, 3, 5)
        jslots = [JL * c + _SLOT_TO_JL[s] for s in range(16)]
        Yc = Ydev[:, jslots, :]                     # (i, 16, 16)
        common45 += _msg45(scal_lw, Yc).sum(axis=1)
    return common45


def _host_finish(common45, aux):
    """common45 (128, 45) f64 perm-independent node features. -> (5,) f32."""
    f = np.float64
    # special to-nodes: exact messages per perm
    z = np.einsum('isb,bh->ish', aux['emb_special'], aux['w1s'])
    hsp = z / (1.0 + np.exp(-z))
    result = np.zeros(5, f)
    na_bias, tp2_w, YS = aux['na_bias'], aux['tp2_w'], aux['YS']
    c2 = np.sqrt(0.2)
    for per in _PERMS:
        node45 = common45.astype(f).copy()
        for si, j in enumerate(SPECIAL):
            Wp = _wsel_perm(aux['features'], aux['fc_w2'], j, per[j])
            scal = np.einsum('ih,hlw->ilw', hsp[:, si], Wp)
            node45 += _msg45(scal, aux['Y_special'][:, si])
        node = np.zeros((N, 50), f)
        node[:, 0:5] = node45[:, 0:5]
        node[:, 10:25] = node45[:, 5:20].reshape(N, 3, 5).swapaxes(1, 2).reshape(N, 15)
        node[:, 25:50] = node45[:, 20:45].reshape(N, 5, 5).swapaxes(1, 2).reshape(N, 25)
        dims = (1, 1, 3, 5)
        offs = (0, 5, 10, 25)
        acts = []
        for bi in range(4):
            xb = node[:, offs[bi]:offs[bi] + MUL * dims[bi]].reshape(N, MUL, dims[bi])
            nrm = np.sqrt(np.sum(xb * xb, -1) + 1e-12)
            scale = 1.0 / (1.0 + np.exp(-(nrm + na_bias[bi * MUL:(bi + 1) * MUL]))) / nrm
            acts.append(xb * scale[..., None])
        for pi, (bidx, l1, l2) in enumerate(_TP2_PATHS):
            A = acts[bidx]
            R = np.einsum('jua,u->ja', A, tp2_w[pi])
            b = 2 * l2 + 1
            result += np.einsum('ja,abk,jb->k', R, _CG[pi],
                                YS[:, _MOFF[l2]:_MOFF[l2] + b]) * c2
    return (result / 24.0).astype(np.float32)


# ---------------------------------------------------------------- runners
_NC_CACHE = {}


def _trn_kernel(pos, features, edge_from, edge_to, fc_w1, fc_w2, tp2_w, na_bias,
                emulate=False):
    in_maps, aux = _host_prep(pos, features, fc_w1, fc_w2, tp2_w, na_bias)
    if emulate:
        scals = [_device_emulate(m) for m in in_maps]
        return _host_finish(_common_from_scals(scals, aux), aux)
    sys.path.insert(0, '/opt/trn_rl_repo')
    from concourse.bass_utils import run_bass_kernel_spmd
    if 'nc' not in _NC_CACHE:
        _NC_CACHE['nc'] = _build_nc()
    nc = _NC_CACHE['nc']
    res = run_bass_kernel_spmd(nc, in_maps, core_ids=list(range(NCORES)))
    scals = [np.asarray(res.results[c]["pout"]) for c in range(NCORES)]
    return _host_finish(_common_from_scals(scals, aux), aux)


def _is_complete_graph(edge_from, edge_to):
    if edge_from.shape != (N * (N - 1),):
        return False
    gi, gj = np.meshgrid(np.arange(N), np.arange(N), indexing='ij')
    m = gi != gj
    return (np.array_equal(np.asarray(edge_from), gi[m].astype(edge_from.dtype))
            and np.array_equal(np.asarray(edge_to), gj[m].astype(edge_to.dtype)))


# ---------------------------------------------------------------- numpy fallback
def _sigmoid(x):
    out = np.empty_like(x)
    p = x >= 0
    out[p] = 1.0 / (1.0 + np.exp(-x[p]))
    ex = np.exp(x[~p])
    out[~p] = ex / (1.0 + ex)
    return out


def _numpy_kernel(pos, features, edge_from, edge_to, fc_w1, fc_w2, tp2_w, na_bias):
    f64 = np.float64
    pos = np.asarray(pos, f64); features = np.asarray(features, f64)
    fc_w1 = np.asarray(fc_w1, f64); fc_w2 = np.asarray(fc_w2, f64)
    tp2_w = np.asarray(tp2_w, f64); na_bias = np.asarray(na_bias, f64)
    E = edge_from.shape[0]
    edge_vec = pos[edge_to] - pos[edge_from]
    d = np.sqrt(np.sum(edge_vec * edge_vec, axis=1))
    u = edge_vec / d[:, None]
    Y = _sh_list(u[:, 0], u[:, 1], u[:, 2])
    vals = np.linspace(0.0, 2.0, BASIS + 2)[1:-1]
    step = 2.0 / (BASIS + 1)
    diff = (d[:, None] - vals) / step

    def f(t):
        tt = np.maximum(t, 1e-8)
        return np.where(t > 0, np.exp(-1.0 / tt), 0.0)

    emb = C_SMOOTH * f(diff + 1.0) * f(1.0 - diff)
    z = emb @ fc_w1 / np.sqrt(BASIS)
    h = ACT_CONST * (z * _sigmoid(z))
    tp_w = (h @ fc_w2 / np.sqrt(H)).reshape(-1, 3, D_IN, MUL)
    eye = np.eye(N, dtype=f64)
    c1 = 1.0 / np.sqrt(D_IN)
    c2 = np.sqrt(0.2)
    dims = (1, 1, 3, 5)
    offs = (0, 5, 10, 25)
    result = np.zeros((5,), dtype=f64)
    for per in _PERMS:
        ext = np.concatenate([features, eye[np.asarray(per)]], axis=1)
        xe = ext[edge_to]
        scal = np.einsum('eluw,eu->elw', tp_w, xe, optimize=True) * c1
        b0 = scal[:, 0, :] * Y[0]
        b1 = (scal[:, 1, :, None] * Y[1][:, None, :]).reshape(-1, MUL * 3)
        b2 = (scal[:, 2, :, None] * Y[2][:, None, :]).reshape(-1, MUL * 5)
        msg = np.concatenate([b0, np.zeros_like(b0), b1, b2], axis=1)
        node = np.zeros((N, 50), dtype=f64)
        np.add.at(node, edge_from, msg)
        acts = []
        for bi in range(4):
            xb = node[:, offs[bi]:offs[bi] + MUL * dims[bi]].reshape(N, MUL, dims[bi])
            nrm = np.sqrt(np.sum(xb * xb, -1) + 1e-12)
            scale = _sigmoid(nrm + na_bias[bi * MUL:(bi + 1) * MUL]) / nrm
            acts.append(xb * scale[..., None])
        out_e = np.zeros((E, 5), dtype=f64)
        for pi, (bidx, l1, l2) in enumerate(_TP2_PATHS):
            A = acts[bidx][edge_to]
            Aw = np.einsum('eui,u->ei', A, tp2_w[pi], optimize=True)
            out_e += np.einsum('ei,ej,ijk->ek', Aw, Y[l2], _CG[pi], optimize=True)
        result += c2 * out_e.sum(axis=0)
    return (result / 24.0).astype(np.float32)


def kernel(pos, features, edge_from, edge_to, fc_w1, fc_w2, tp2_w, na_bias):
    edge_from = np.asarray(edge_from)
    edge_to = np.asarray(edge_to)
    if _is_complete_graph(edge_from, edge_to):
        try:
            return _trn_kernel(pos, features, edge_from, edge_to,
                               fc_w1, fc_w2, tp2_w, na_bias)
        except Exception as e:  # pragma: no cover - safety net
            print(f"[kernel] TRN path failed ({type(e).__name__}: {e}); "
                  f"falling back to numpy", file=sys.stderr)
    return _numpy_kernel(pos, features, edge_from, edge_to,
                         fc_w1, fc_w2, tp2_w, na_bias)


# revision 40
# speedup vs baseline: 1.0257x; 1.0257x over previous
"""Trainium2 Bass kernel for nn_Polynomial_91259465105963 (gnn_message_passing).

8 NeuronCores, to-sharded: core c owns to-nodes J_c=[16c,16c+16). Key
structure exploited:
  * complete graph + one-hot features collapse tp1 into per-to-node
    (50 -> 15) matmuls;
  * the 5 permutations are the first 5 lex perms of range(127): they differ
    ONLY at positions {124,125,126}. So the device computes a single
    perm-INDEPENDENT pass (identity weights); the three perm-varying
    to-nodes are handled exactly on the host in f64 (their emb columns are
    zeroed on device, their Y rows zeroed in the host readout).
  * fp16 single-pass matmuls everywhere (PE runs fp16 at bf16 rate, f32
    accumulation in PSUM); emulated end-to-end rel-err ~5e-4 vs tolerance
    2e-2 -- no hi/lo splits needed.
  * the device stops at scal = h @ W (the per-edge tp1 coefficients): the
    Y-multiply + slot reduction + NormActivation + tp2 readout are only
    ~1.5 MFLOP, done on host in f64. On-device those cost ~2.7us of
    DVE/Pool serial chain; shipping scal (128x256 fp16, 64KB) is faster.

Device pipeline per core (~20 engine instructions, measured ~16.8us where
~13.7us is fixed framework preamble/exit-barriers and DMA latency):
  DMAs: estack halves on two queues (Sync + ACT) so the two A matmuls run
     concurrently; W on GpSimd.
  A: z = w1^T emb as 2 block-diagonal matmuls (M=100: two j-halves stacked
     on output partitions; the second placed in PE row-tiles 64-103 via
     tile_position so it overlaps the first). PSUM z (100, 1024).
  silu: 1 ACT op using the hardware Silu table -> h fp16 (100, 1024).
  B: 8 pair matmuls, lhsT = h (100, 128) slice, rhs = block-diag W
     (100, 32) -> scal PSUM (128, 16 slots x 16ch).
  out: ACT copies scal -> fp16 SBUF, GpSimd DMAs it to HBM.
Host: per-core Y-multiply + slot sum (f64), adds the 3 special to-nodes'
messages per perm, NormActivation + tp2 readout.
"""
import sys
import numpy as np
from itertools import permutations, islice

N = 128
BASIS = 20
MUL = 5
H = 50
D_IN = N + 1
ACT_CONST = 1.6790
C_SMOOTH = 1.14136 * float(np.exp(2.0))
NCORES = 8
JL = N // NCORES              # 16 to-nodes per core
NPAIR = JL // 2               # 8 pair-matmuls
SPECIAL = (124, 125, 126)     # to-nodes whose weights vary across perms

USE_SILU_TABLE = True         # False -> tanh table + DVE stt fallback

_TP2_PATHS = [(0, 0, 2), (2, 1, 1), (2, 1, 3), (3, 2, 0), (3, 2, 2)]
_MOFF = (0, 1, 4, 9)
_MDIM = (1, 3, 5)


def _sh_list(x, y, z):
    s3, s5, s7 = np.sqrt(3.0), np.sqrt(5.0), np.sqrt(7.0)
    s15, s42, s70, s105 = np.sqrt(15.0), np.sqrt(42.0), np.sqrt(70.0), np.sqrt(105.0)
    one = np.ones_like(x)
    y0 = np.stack([one], -1)
    y1 = np.stack([s3 * y, s3 * z, s3 * x], -1)
    y2 = np.stack([s15 * x * y, s15 * y * z, 0.5 * s5 * (3 * z * z - 1.0),
                   s15 * x * z, 0.5 * s15 * (x * x - y * y)], -1)
    y3 = np.stack([0.25 * s70 * y * (3 * x * x - y * y), s105 * x * y * z,
                   0.25 * s42 * y * (5 * z * z - 1.0), 0.5 * s7 * z * (5 * z * z - 3.0),
                   0.25 * s42 * x * (5 * z * z - 1.0), 0.5 * s105 * z * (x * x - y * y),
                   0.25 * s70 * x * (x * x - 3 * y * y)], -1)
    return [y0, y1, y2, y3]


def _gaunt(l1, l2, l3):
    zq, wq = np.polynomial.legendre.leggauss(20)
    nphi = 48
    phi = 2 * np.pi * np.arange(nphi) / nphi
    Z = np.repeat(zq[:, None], nphi, 1)
    P = np.broadcast_to(phi, Z.shape)
    W = np.repeat(wq[:, None], nphi, 1) * (2 * np.pi / nphi)
    st = np.sqrt(np.clip(1.0 - Z * Z, 0.0, None))
    Y = _sh_list(st * np.cos(P), st * np.sin(P), Z)
    G = np.einsum('ab,abi,abj,abk->ijk', W, Y[l1], Y[l2], Y[l3])
    return (G / np.linalg.norm(G)).astype(np.float64)


_CG = [_gaunt(l1, l2, 2) for (_, l1, l2) in _TP2_PATHS]
_PERMS = [list(p) + [N - 1] for p in islice(permutations(range(N - 1)), 5)]

# slot s (0..15) within a core <-> local to-node jl: pair k = s//2,
# half b = s&1 -> jl = k + 8*b. Matches the B-matmul scal column order.
_SLOT_TO_JL = [s // 2 + 8 * (s & 1) for s in range(16)]


# ---------------------------------------------------------------- host prep
def _geom(pos):
    """Per-(i,j) geometry in f64: Ygrid (i,j,16), emb (i,j,20), diag zeroed."""
    f = np.float64
    pos = np.asarray(pos, f)
    dvec = pos[None, :, :] - pos[:, None, :]          # pos[to] - pos[from]
    d2 = np.sum(dvec * dvec, axis=-1)
    np.fill_diagonal(d2, 1.0)
    d = np.sqrt(d2)
    u = dvec / d[..., None]
    Yl = _sh_list(u[..., 0], u[..., 1], u[..., 2])
    Ygrid = np.concatenate(Yl, axis=-1)               # (i, j, 16)
    mask = 1.0 - np.eye(N)
    Ygrid *= mask[:, :, None]
    vals = np.linspace(0.0, 2.0, BASIS + 2)[1:-1]
    step = 2.0 / (BASIS + 1)
    q = (d[..., None] - vals) / step
    g = 1.0 - q * q
    with np.errstate(divide='ignore', over='ignore'):
        emb = np.where(g > 0, np.exp(-2.0 / np.maximum(g, 1e-30)), 0.0) * C_SMOOTH
    emb *= mask[:, :, None]
    return Ygrid, emb


def _wsel_identity(features, fc_w2):
    """Identity-perm weights W[j] (H, 3, 5) incl. tp1 norm + silu consts."""
    f = np.float64
    W2 = np.asarray(fc_w2, f).reshape(H, 3, D_IN, MUL)
    c = (1.0 / np.sqrt(D_IN)) * ACT_CONST / np.sqrt(H)
    if not USE_SILU_TABLE:
        c *= 0.5                      # tanh path: silu = 0.5*z*(1+tanh(z/2))
    A0 = W2[:, :, 0, :]
    feats = np.asarray(features, f)[:, 0]
    Wj = A0[None] * feats[:, None, None, None] + np.moveaxis(W2[:, :, 1:, :], 2, 0)
    return Wj * c                     # (j, H, 3, 5)


def _wsel_perm(features, fc_w2, j, perm_j):
    f = np.float64
    W2 = np.asarray(fc_w2, f).reshape(H, 3, D_IN, MUL)
    c = (1.0 / np.sqrt(D_IN)) * ACT_CONST / np.sqrt(H)
    A0 = W2[:, :, 0, :]
    return (A0 * float(np.asarray(features, f)[j, 0]) + W2[:, :, 1 + perm_j, :]) * c


def _host_prep(pos, features, fc_w1, fc_w2, tp2_w, na_bias):
    f16, f32 = np.float16, np.float32
    Ygrid, emb = _geom(pos)
    w1s = np.asarray(fc_w1, np.float64) / np.sqrt(BASIS)
    Wj = _wsel_identity(features, fc_w2)

    emb_dev = emb.copy()
    emb_dev[:, SPECIAL, :] = 0.0      # specials handled on host

    # w1 block-diagonal (40, 100)
    w1bd = np.zeros((2 * BASIS, 2 * H), np.float64)
    w1bd[0:BASIS, 0:H] = w1s
    w1bd[BASIS:2 * BASIS, H:2 * H] = w1s

    in_maps = []
    for c in range(NCORES):
        jbase = JL * c
        # ---- estack (80, 612): two 40-row blocks (SBUF rows 0-39 / 64-103).
        # Per block: cols 0-99 = w1bd, cols 100-611 = emb columns for pairs
        # 0-3 (block 0) / 4-7 (block 1), col = 100 + 128*(k%4) + i.
        estack = np.zeros((80, 612), f16)
        estack[0:40, 0:100] = w1bd.astype(f16)
        estack[40:80, 0:100] = w1bd.astype(f16)
        for k in range(NPAIR):
            ja = jbase + k
            jb = jbase + k + 8
            rbase = 0 if k < 4 else 40
            cbase = 100 + 128 * (k % 4)
            # emb[i, j, b20] -> rows of estack (basis down rows)
            estack[rbase:rbase + 20, cbase:cbase + 128] = \
                emb_dev[:, ja, :].T.astype(f16)
            estack[rbase + 20:rbase + 40, cbase:cbase + 128] = \
                emb_dev[:, jb, :].T.astype(f16)

        # ---- wstack (100, 256): per pair k cols 32k..32k+32, block-diag
        wstack = np.zeros((100, 256), f16)
        Wflat = Wj.reshape(N, H, 15)
        for k in range(NPAIR):
            ja = jbase + k
            jb = jbase + k + 8
            wstack[0:50, 32 * k:32 * k + 15] = Wflat[ja].astype(f16)
            wstack[50:100, 32 * k + 16:32 * k + 31] = Wflat[jb].astype(f16)

        in_maps.append(dict(
            estack=np.ascontiguousarray(estack),
            wstack=np.ascontiguousarray(wstack),
        ))

    Ydev = Ygrid.copy()
    Ydev[:, SPECIAL, :] = 0.0                       # specials excluded on host
    aux = dict(
        YS=Ygrid.sum(axis=0),                       # (j, 16)
        Ydev=Ydev,                                  # (i, j, 16)
        na_bias=np.asarray(na_bias, np.float64),
        tp2_w=np.asarray(tp2_w, np.float64),
        w1s=w1s,
        features=np.asarray(features, np.float64),
        fc_w2=np.asarray(fc_w2, np.float64),
        emb_special=emb[:, SPECIAL, :],             # (i, 3, 20) exact
        Y_special=Ygrid[:, SPECIAL, :],             # (i, 3, 16)
    )
    return in_maps, aux


# ---------------------------------------------------------------- device emu
def _device_emulate(in_map):
    """Numpy emulation of the device program for one core (fp16 rounding at
    the same places). Returns scal (128, 256) fp16."""
    f16, f32 = np.float16, np.float32
    estack = in_map['estack'].astype(f32)
    wstack = in_map['wstack'].astype(f32)
    # A: two block-diag matmuls (f32 accumulation of fp16 operands)
    z = np.zeros((100, 1024), f32)
    z[:, 0:512] = estack[0:40, 0:100].T @ estack[0:40, 100:612]
    z[:, 512:1024] = estack[40:80, 0:100].T @ estack[40:80, 100:612]
    h = (z / (1.0 + np.exp(-z.astype(np.float64)))).astype(f16).astype(f32)
    # B: 8 pair matmuls -> scal (128, 256)
    scal = np.zeros((N, 256), f32)
    for k in range(NPAIR):
        scal[:, 32 * k:32 * (k + 1)] = h[:, 128 * k:128 * (k + 1)].T @ wstack[:, 32 * k:32 * (k + 1)]
    return scal.astype(f16)


# ---------------------------------------------------------------- bass build
def _build_nc():
    sys.path.insert(0, '/opt/trn_rl_repo')
    import concourse.bass as bass  # noqa: F401
    import concourse.tile as tile
    from concourse import bacc, mybir

    dt = mybir.dt
    f32, f16 = dt.float32, dt.float16
    Alu = mybir.AluOpType
    Act = mybir.ActivationFunctionType

    nc = bacc.Bacc("TRN2", target_bir_lowering=False, debug=False,
                   num_devices=NCORES)
    es_d = nc.dram_tensor("estack", [80, 612], f16, kind="ExternalInput").ap()
    ws_d = nc.dram_tensor("wstack", [100, 256], f16, kind="ExternalInput").ap()
    out_d = nc.dram_tensor("pout", [N, 256], f16, kind="ExternalOutput").ap()

    with tile.TileContext(nc) as tc:
        with tc.tile_pool(name="sb", bufs=1) as sb, \
             tc.tile_pool(name="ps", bufs=1, space="PSUM") as ps:

            # ---- input DMAs. estack halves go to two different queues so
            # both transfers land together and the two A matmuls (distinct
            # PE row-tiles: block 2 lives at SBUF partitions 64-103) run
            # concurrently.
            es = sb.tile([104, 612], f16)
            nc.sync.dma_start(es[0:40, :], es_d[0:40, :])
            nc.scalar.dma_start(es[64:104, :], es_d[40:80, :])
            ws = sb.tile([100, 256], f16)
            nc.gpsimd.dma_start(ws[:], ws_d)

            # ---- A: z = w1bd^T @ emb, two matmuls in distinct PE row-tiles
            zps = ps.tile([100, 1024], f32, tag="zmm")
            nc.tensor.matmul(zps[:, 0:512], es[0:40, 0:100],
                             es[0:40, 100:612], start=True, stop=True)
            nc.tensor.matmul(zps[:, 512:1024], es[64:104, 0:100],
                             es[64:104, 100:612], start=True, stop=True,
                             tile_position=(64, 0))

            # ---- silu -> h fp16 (100, 1024). One ACT op measured faster
            # than two chunked ones: the A matmuls land only ~0.3us apart,
            # so chunking's extra per-op overhead (~2x390 cycles of
            # PSUM/SBUF access setup) exceeds the overlap it buys.
            h = sb.tile([100, 1024], f16)
            if USE_SILU_TABLE:
                nc.scalar.activation(h[:], zps[:], Act.Silu)
            else:
                t1 = sb.tile([100, 1024], f32, tag="t1")
                nc.scalar.activation(t1[:], zps[:], Act.Tanh, scale=0.5)
                nc.vector.scalar_tensor_tensor(h[:], t1[:], 1.0,
                                               zps[:], Alu.add, Alu.mult)

            # ---- B: 8 pair matmuls -> scal PSUM (128, 16 slots x 16 ch)
            sps = ps.tile([N, 256], f32, tag="smm")
            for k in range(NPAIR):
                nc.tensor.matmul(sps[:, 32 * k:32 * (k + 1)],
                                 h[:, 128 * k:128 * (k + 1)],
                                 ws[:, 32 * k:32 * (k + 1)],
                                 start=True, stop=True)

            # ---- ship scal (fp16) straight to the host: the Y-multiply +
            # slot reduction is only ~1.5 MFLOP there, while on-device it
            # costs ~2.7us of DVE/Pool serial chain. PSUM can't be DMA'd,
            # so copy to fp16 SBUF -- split across ACT and the otherwise
            # idle DVE so the halves run in parallel.
            # Single copy + same-engine (ACT) DMA issue: no cross-engine
            # semaphore hop anywhere on the exit path. (Splitting the copy
            # measured slower -- per-op overhead again.)
            sc = sb.tile([N, 256], f16)
            nc.scalar.copy(sc[:], sps[:])
            nc.scalar.dma_start(out_d, sc[:])
    nc.compile()
    return nc


# ---------------------------------------------------------------- host finish
def _msg45(scal_lw, Y16):
    """scal_lw (..., 3, 5), Y16 (..., 16) -> (..., 45) [l0 5][l1 15][l2 25]."""
    b0 = scal_lw[..., 0, :] * Y16[..., 0:1]
    b1 = (Y16[..., 1:4, None] * scal_lw[..., None, 1, :]).reshape(*scal_lw.shape[:-2], 15)
    b2 = (Y16[..., 4:9, None] * scal_lw[..., None, 2, :]).reshape(*scal_lw.shape[:-2], 25)
    return np.concatenate([b0, b1, b2], axis=-1)


def _common_from_scals(scals, aux):
    """scals: list of 8 (128, 256) arrays (fp16 from device). Returns the
    perm-independent node features common45 (128, 45) f64."""
    f = np.float64
    Ydev = aux['Ydev']
    common45 = np.zeros((N, 45), f)
    for c in range(NCORES):
        scal_lw = np.asarray(scals[c], f).reshape(N, 16, 16)[:, :, 0:15] \
            .reshape(N, 16, 3, 5)
        jslots = [JL * c + _SLOT_TO_JL[s] for s in range(16)]
        Yc = Ydev[:, jslots, :]                     # (i, 16, 16)
        common45 += _msg45(scal_lw, Yc).sum(axis=1)
    return common45


def _host_finish(common45, aux):
    """common45 (128, 45) f64 perm-independent node features. -> (5,) f32."""
    f = np.float64
    # special to-nodes: exact messages per perm
    z = np.einsum('isb,bh->ish', aux['emb_special'], aux['w1s'])
    hsp = z / (1.0 + np.exp(-z))
    result = np.zeros(5, f)
    na_bias, tp2_w, YS = aux['na_bias'], aux['tp2_w'], aux['YS']
    c2 = np.sqrt(0.2)
    for per in _PERMS:
        node45 = common45.astype(f).copy()
        for si, j in enumerate(SPECIAL):
            Wp = _wsel_perm(aux['features'], aux['fc_w2'], j, per[j])
            scal = np.einsum('ih,hlw->ilw', hsp[:, si], Wp)
            node45 += _msg45(scal, aux['Y_special'][:, si])
        node = np.zeros((N, 50), f)
        node[:, 0:5] = node45[:, 0:5]
        node[:, 10:25] = node45[:, 5:20].reshape(N, 3, 5).swapaxes(1, 2).reshape(N, 15)
        node[:, 25:50] = node45[:, 20:45].reshape(N, 5, 5).swapaxes(1, 2).reshape(N, 25)
        dims = (1, 1, 3, 5)
        offs = (0, 5, 10, 25)
        acts = []
        for bi in range(4):
            xb = node[:, offs[bi]:offs[bi] + MUL * dims[bi]].reshape(N, MUL, dims[bi])
            nrm = np.sqrt(np.sum(xb * xb, -1) + 1e-12)
            scale = 1.0 / (1.0 + np.exp(-(nrm + na_bias[bi * MUL:(bi + 1) * MUL]))) / nrm
            acts.append(xb * scale[..., None])
        for pi, (bidx, l1, l2) in enumerate(_TP2_PATHS):
            A = acts[bidx]
            R = np.einsum('jua,u->ja', A, tp2_w[pi])
            b = 2 * l2 + 1
            result += np.einsum('ja,abk,jb->k', R, _CG[pi],
                                YS[:, _MOFF[l2]:_MOFF[l2] + b]) * c2
    return (result / 24.0).astype(np.float32)


# ---------------------------------------------------------------- runners
_NC_CACHE = {}


def _trn_kernel(pos, features, edge_from, edge_to, fc_w1, fc_w2, tp2_w, na_bias,
                emulate=False):
    in_maps, aux = _host_prep(pos, features, fc_w1, fc_w2, tp2_w, na_bias)
    if emulate:
        scals = [_device_emulate(m) for m in in_maps]
        return _host_finish(_common_from_scals(scals, aux), aux)
    sys.path.insert(0, '/opt/trn_rl_repo')
    from concourse.bass_utils import run_bass_kernel_spmd
    if 'nc' not in _NC_CACHE:
        _NC_CACHE['nc'] = _build_nc()
    nc = _NC_CACHE['nc']
    res = run_bass_kernel_spmd(nc, in_maps, core_ids=list(range(NCORES)))
    scals = [np.asarray(res.results[c]["pout"]) for c in range(NCORES)]
    return _host_finish(_common_from_scals(scals, aux), aux)


def _is_complete_graph(edge_from, edge_to):
    if edge_from.shape != (N * (N - 1),):
        return False
    gi, gj = np.meshgrid(np.arange(N), np.arange(N), indexing='ij')
    m = gi != gj
    return (np.array_equal(np.asarray(edge_from), gi[m].astype(edge_from.dtype))
            and np.array_equal(np.asarray(edge_to), gj[m].astype(edge_to.dtype)))


# ---------------------------------------------------------------- numpy fallback
def _sigmoid(x):
    out = np.empty_like(x)
    p = x >= 0
    out[p] = 1.0 / (1.0 + np.exp(-x[p]))
    ex = np.exp(x[~p])
    out[~p] = ex / (1.0 + ex)
    return out


def _numpy_kernel(pos, features, edge_from, edge_to, fc_w1, fc_w2, tp2_w, na_bias):
    f64 = np.float64
    pos = np.asarray(pos, f64); features = np.asarray(features, f64)
    fc_w1 = np.asarray(fc_w1, f64); fc_w2 = np.asarray(fc_w2, f64)
    tp2_w = np.asarray(tp2_w, f64); na_bias = np.asarray(na_bias, f64)
    E = edge_from.shape[0]
    edge_vec = pos[edge_to] - pos[edge_from]
    d = np.sqrt(np.sum(edge_vec * edge_vec, axis=1))
    u = edge_vec / d[:, None]
    Y = _sh_list(u[:, 0], u[:, 1], u[:, 2])
    vals = np.linspace(0.0, 2.0, BASIS + 2)[1:-1]
    step = 2.0 / (BASIS + 1)
    diff = (d[:, None] - vals) / step

    def f(t):
        tt = np.maximum(t, 1e-8)
        return np.where(t > 0, np.exp(-1.0 / tt), 0.0)

    emb = C_SMOOTH * f(diff + 1.0) * f(1.0 - diff)
    z = emb @ fc_w1 / np.sqrt(BASIS)
    h = ACT_CONST * (z * _sigmoid(z))
    tp_w = (h @ fc_w2 / np.sqrt(H)).reshape(-1, 3, D_IN, MUL)
    eye = np.eye(N, dtype=f64)
    c1 = 1.0 / np.sqrt(D_IN)
    c2 = np.sqrt(0.2)
    dims = (1, 1, 3, 5)
    offs = (0, 5, 10, 25)
    result = np.zeros((5,), dtype=f64)
    for per in _PERMS:
        ext = np.concatenate([features, eye[np.asarray(per)]], axis=1)
        xe = ext[edge_to]
        scal = np.einsum('eluw,eu->elw', tp_w, xe, optimize=True) * c1
        b0 = scal[:, 0, :] * Y[0]
        b1 = (scal[:, 1, :, None] * Y[1][:, None, :]).reshape(-1, MUL * 3)
        b2 = (scal[:, 2, :, None] * Y[2][:, None, :]).reshape(-1, MUL * 5)
        msg = np.concatenate([b0, np.zeros_like(b0), b1, b2], axis=1)
        node = np.zeros((N, 50), dtype=f64)
        np.add.at(node, edge_from, msg)
        acts = []
        for bi in range(4):
            xb = node[:, offs[bi]:offs[bi] + MUL * dims[bi]].reshape(N, MUL, dims[bi])
            nrm = np.sqrt(np.sum(xb * xb, -1) + 1e-12)
            scale = _sigmoid(nrm + na_bias[bi * MUL:(bi + 1) * MUL]) / nrm
            acts.append(xb * scale[..., None])
        out_e = np.zeros((E, 5), dtype=f64)
        for pi, (bidx, l1, l2) in enumerate(_TP2_PATHS):
            A = acts[bidx][edge_to]
            Aw = np.einsum('eui,u->ei', A, tp2_w[pi], optimize=True)
            out_e += np.einsum('ei,ej,ijk->ek', Aw, Y[l2], _CG[pi], optimize=True)
        result += c2 * out_e.sum(axis=0)
    return (result / 24.0).astype(np.float32)


def kernel(pos, features, edge_from, edge_to, fc_w1, fc_w2, tp2_w, na_bias):
    edge_from = np.asarray(edge_from)
    edge_to = np.asarray(edge_to)
    if _is_complete_graph(edge_from, edge_to):
        try:
            return _trn_kernel(pos, features, edge_from, edge_to,
                               fc_w1, fc_w2, tp2_w, na_bias)
        except Exception as e:  # pragma: no cover - safety net
            print(f"[kernel] TRN path failed ({type(e).__name__}: {e}); "
                  f"falling back to numpy", file=sys.stderr)
    return _numpy_kernel(pos, features, edge_from, edge_to,
                         fc_w1, fc_w2, tp2_w, na_bias)
